# revision 23
# baseline (speedup 1.0000x reference)
"""ArcConceptDecoder (work=True inference path) as a distributed Bass kernel
on 8 TRN2 NeuronCores.

Sharding: batch (B=16) split as one b-pair per core (b = 2*core, 2*core+1).
The scatter_add is row-local per (src, b) and its indices (copy_seq) are known
at kernel-build time, so the scatter is realized as per-chunk compact one-hot
matmuls on TensorE with a structure that is uniform across cores (padded to a
global per-chunk row capacity CAP).

Per core, per b:
  hT[C,S]      = tanh(WtT.T @ coT)                  (bf16, TensorE+ACT)
  gates        = softmax(h @ Wd.T) row-major (gen gate) and transposed (2 rows)
  cpT[j,S]     = alignP (host-sorted/padded rows of align.T) * gate_perm
  x[S,1000]    = hT.T @ WgT chunks (PSUM), exp -> e (bf16) with accum -> Z
  scat[S,500]  = cpT_chunk.T @ onehot_chunk (PSUM)
  ll           = log((gen/Z) * e + scat + 1e-12)    (STT on DVE + log on ACT)
  ext chunk    = log(scat + 1e-12)
  arc_ll       = log(arc + 1e-12)
"""

import numpy as np
import ml_dtypes

import concourse.bass as bass
import concourse.mybir as mybir
import concourse.tile as tile
from concourse.bass_utils import run_bass_kernel_spmd

BF16 = ml_dtypes.bfloat16

SRC, B, SNT, E, C, V, EXT = 512, 16, 512, 512, 512, 12000, 500
TOT = V + EXT            # 12500
CH = 500                 # output chunk width (PSUM bank holds 512 f32)
NCH = TOT // CH          # 25 chunks; chunk 24 is the ext (scatter-only) region
GRP = 1000               # base-matmul group width (2 chunks per PSUM group)
NGRP = V // GRP          # 12 groups of the softmax region
NB = 2                   # batch entries per core
NCORES = 8
EPS = 1e-12


# ---------------------------------------------------------------- host tables
def _host_tables(copy_seq):
    """Sorted scatter tables per b + global row capacity CAP per chunk."""
    copy_seq = np.asarray(copy_seq)[:SNT]     # [SNT, B, 2]
    per_b = []
    max_cnt = 0
    for b in range(B):
        tgt = np.concatenate([copy_seq[:, b, 0], copy_seq[:, b, 1]]).astype(np.int64)
        par = np.concatenate([np.zeros(SNT, np.int64), np.ones(SNT, np.int64)])
        t_idx = np.concatenate([np.arange(SNT), np.arange(SNT)])
        order = np.argsort(tgt, kind="stable")
        tgt_s, par_s, t_s = tgt[order], par[order], t_idx[order]
        chunk = tgt_s // CH
        pos = (tgt_s % CH).astype(np.int64)
        starts = np.searchsorted(chunk, np.arange(NCH))
        ends = np.searchsorted(chunk, np.arange(NCH), side="right")
        max_cnt = max(max_cnt, int((ends - starts).max()))
        per_b.append((t_s, par_s, pos, starts, ends))
    cap = ((max_cnt + 31) // 32) * 32
    return per_b, cap


def _host_prep(inputs):
    aw = np.asarray(inputs["alignment_weight"], np.float32)   # [S, B, SNT]
    arc = np.asarray(inputs["arc_weight"], np.float32)        # [S, B, S]
    co = np.asarray(inputs["concept_outs"], np.float32)       # [S, B, E]
    Wt = np.asarray(inputs["W_transfer"], np.float32)         # [C, E]
    Wg = np.asarray(inputs["W_gen"], np.float32)              # [V, C]
    Wd = np.asarray(inputs["W_div"], np.float32)              # [3, C]
    for nm in ("b_transfer", "b_gen", "b_div"):
        assert not np.asarray(inputs[nm]).any(), f"{nm} must be zero"

    per_b, cap = _host_tables(inputs["copy_seq"])

    WtT = np.ascontiguousarray(Wt.T).astype(BF16).reshape(4, 128, C)
    WdT = np.ascontiguousarray(Wd.T).astype(BF16).reshape(4, 128, 3)
    WgT = np.ascontiguousarray(Wg.T).astype(ml_dtypes.float8_e4m3).reshape(4, 128, V)

    in_maps = []
    for core in range(NCORES):
        bs = [NB * core + i for i in range(NB)]
        coT = np.stack(
            [np.ascontiguousarray(co[:, b, :].T).astype(BF16).reshape(4, 128, SRC)
             for b in bs])                                     # [NB,4,128,S]
        alignP = np.zeros((NB, NCH, cap, SRC), BF16)
        onehotP = np.zeros((NB, NCH, cap, CH), BF16)
        parsel = np.zeros((NB, 2, NCH * cap), BF16)
        for i, b in enumerate(bs):
            t_s, par_s, pos, starts, ends = per_b[b]
            awT = aw[:, b, :].T.astype(BF16)                   # [SNT, S]
            for c in range(NCH):
                s0, s1 = int(starts[c]), int(ends[c])
                n = s1 - s0
                if n == 0:
                    continue
                alignP[i, c, :n] = awT[t_s[s0:s1]]
                onehotP[i, c, np.arange(n), pos[s0:s1]] = BF16(1.0)
                # gate row selector: row0 = map gate (par=1), row1 = copy (par=0)
                sel_row = np.where(par_s[s0:s1] == 0, 1, 0)
                parsel[i, sel_row, c * cap + np.arange(n)] = BF16(1.0)
        arc_s = np.ascontiguousarray(arc[:, bs, :])            # [S, NB, S]
        in_maps.append(dict(
            coT=coT, WtT=WtT, WdT=WdT, WgT=WgT,
            alignP=alignP, onehotP=onehotP, parsel=parsel, arc=arc_s,
        ))
    return in_maps, cap


# ---------------------------------------------------------------- bass kernel
def _split_multi_waits(nc):
    """This walrus build only accepts one sync-wait per instruction: split
    multi-wait instructions by hoisting extra waits onto same-engine drains."""
    n = 0
    for f in nc.m.functions:
        for blk in f.blocks:
            new_insts = []
            for inst in blk.instructions:
                si = inst.sync_info
                if si is not None and si.on_wait and len(si.on_wait) > 1:
                    waits = list(si.on_wait)
                    for w in waits[:-1]:
                        nop = mybir.InstDrain(name=f"WSPLIT-{n}", ins=[], outs=[])
                        nop.engine = inst.engine
                        nop.sync_info = mybir.SyncInfo(on_wait=[w], on_update=[])
                        new_insts.append(nop)
                        n += 1
                    inst.sync_info = mybir.SyncInfo(
                        on_wait=[waits[-1]], on_update=list(si.on_update))
                new_insts.append(inst)
            blk.instructions = new_insts
    return n


def _build_bass(cap, knobs=None):
    kn = dict(arc_early=False, cpt_bufs=1, e_bufs=4, px_bufs=4, psc_bufs=2,
              oh_bufs=8, stg_bufs=2, alg_bufs=4, hT_bufs=2)
    if knobs:
        kn.update(knobs)
    fp32 = mybir.dt.float32
    bf16 = mybir.dt.bfloat16
    AF = mybir.ActivationFunctionType
    ALU = mybir.AluOpType
    nc = bass.Bass()

    coT_d = nc.declare_dram_parameter("coT", [NB, 4, 128, SRC], bf16, isOutput=False)
    WtT_d = nc.declare_dram_parameter("WtT", [4, 128, C], bf16, isOutput=False)
    WdT_d = nc.declare_dram_parameter("WdT", [4, 128, 3], bf16, isOutput=False)
    WgT_d = nc.declare_dram_parameter("WgT", [4, 128, V], mybir.dt.float8e4, isOutput=False)
    alignP_d = nc.declare_dram_parameter("alignP", [NB, NCH, cap, SRC], bf16, isOutput=False)
    onehotP_d = nc.declare_dram_parameter("onehotP", [NB, NCH, cap, CH], bf16, isOutput=False)
    parsel_d = nc.declare_dram_parameter("parsel", [NB, 2, NCH * cap], bf16, isOutput=False)
    arc_d = nc.declare_dram_parameter("arc", [SRC, NB, SRC], fp32, isOutput=False)
    ll_d = nc.declare_dram_parameter("ll", [SRC, NB, TOT], fp32, isOutput=True)
    arcll_d = nc.declare_dram_parameter("arcll", [SRC, NB, SRC], fp32, isOutput=True)

    with tile.TileContext(nc) as tc:
        with (
            tc.tile_pool(name="wg", bufs=1) as wg_pool,
            tc.tile_pool(name="wsm", bufs=1) as wsm_pool,
            tc.tile_pool(name="hT", bufs=kn["hT_bufs"]) as hT_pool,
            tc.tile_pool(name="cpT", bufs=kn["cpt_bufs"]) as cpT_pool,
            tc.tile_pool(name="oh", bufs=kn["oh_bufs"]) as oh_pool,
            tc.tile_pool(name="alg", bufs=kn["alg_bufs"]) as alg_pool,
            tc.tile_pool(name="e", bufs=kn["e_bufs"]) as e_pool,
            tc.tile_pool(name="stg", bufs=kn["stg_bufs"]) as stg_pool,
            tc.tile_pool(name="arcp", bufs=2) as arc_pool,
            tc.tile_pool(name="sml", bufs=2) as sml_pool,
            tc.tile_pool(name="px", bufs=kn["px_bufs"], space="PSUM") as px_pool,
            tc.tile_pool(name="psc", bufs=kn["psc_bufs"], space="PSUM") as psc_pool,
            tc.tile_pool(name="psm", bufs=1, space="PSUM") as psm_pool,
        ):
            epsb = wsm_pool.tile([128, 1], fp32, name="epsb", tag="epsb")
            nc.vector.memset(epsb[:], EPS)

            def emit_arc():
                for b in range(NB):
                    for st in range(4):
                        a_in = arc_pool.tile([128, SRC], fp32, name="a_in", tag="a_in")
                        nc.sync.dma_start(a_in[:], arc_d[st * 128:(st + 1) * 128, b, :])
                        a_out = arc_pool.tile([128, SRC], fp32, name="a_out", tag="a_out")
                        nc.scalar.activation(a_out[:], a_in[:], AF.Ln, bias=epsb[:])
                        nc.sync.dma_start(arcll_d[st * 128:(st + 1) * 128, b, :], a_out[:])

            if kn["arc_early"]:
                emit_arc()

            # ---- resident weights
            wg_t = []
            for kt in range(4):
                w = wg_pool.tile([128, V], mybir.dt.float8e4, name=f"wg{kt}", tag=f"wg{kt}")
                nc.sync.dma_start(w[:], WgT_d[kt])
                wg_t.append(w)
            wt_t, wd_t = [], []
            for kt in range(4):
                w1 = wsm_pool.tile([128, C], bf16, name=f"wt{kt}", tag=f"wt{kt}")
                nc.sync.dma_start(w1[:], WtT_d[kt])
                wt_t.append(w1)
                w2 = wsm_pool.tile([128, 3], bf16, name=f"wd{kt}", tag=f"wd{kt}")
                nc.sync.dma_start(w2[:], WdT_d[kt])
                wd_t.append(w2)
            ones3 = wsm_pool.tile([3, 1], fp32, name="ones3", tag="ones3")
            nc.vector.memset(ones3[:], 1.0)
            ones12 = wsm_pool.tile([1, 2], fp32, name="ones12", tag="ones12")
            nc.vector.memset(ones12[:], 1.0)

            for b in range(NB):
                # ---- hT = tanh(WtT.T @ coT): [C(4x128), S] bf16
                co_t = []
                for et in range(4):
                    cot = sml_pool.tile([128, SRC], bf16, name="cot", tag="cot", bufs=4)
                    nc.sync.dma_start(cot[:], coT_d[b, et])
                    co_t.append(cot)
                hT = []
                for ct in range(4):
                    hps = px_pool.tile([128, 512], fp32, name="hps", tag="px", bufs=kn["px_bufs"])
                    for et in range(4):
                        nc.tensor.matmul(
                            hps[:],
                            wt_t[et][:, ct * 128:(ct + 1) * 128],
                            co_t[et][:],
                            start=(et == 0), stop=(et == 3))
                    ht = hT_pool.tile([128, SRC], bf16, name="ht", tag=f"ht{ct}")
                    nc.scalar.activation(ht[:], hps[:], AF.Tanh)
                    hT.append(ht)

                # ---- transposed gate rows g2rows [2, S]: row0=map, row1=copy
                glT = psm_pool.tile([3, SRC], fp32, name="glT", tag="psm")
                for ct in range(4):
                    nc.tensor.matmul(glT[:], wd_t[ct][:], hT[ct][:],
                                     start=(ct == 0), stop=(ct == 3))
                egT = sml_pool.tile([3, SRC], fp32, name="egT", tag="egT", bufs=1)
                nc.scalar.activation(egT[:], glT[:], AF.Exp)
                zg3 = psm_pool.tile([1, SRC], fp32, name="zg3", tag="psm")
                nc.tensor.matmul(zg3[:], ones3[:], egT[:], start=True, stop=True)
                rcp3 = sml_pool.tile([1, SRC], fp32, name="rcp3", tag="rcp3", bufs=1)
                nc.vector.reciprocal(rcp3[:], zg3[:])
                # [2, S] psum of the (map, copy) logit rows -> exp -> * 1/Z
                glT2 = psm_pool.tile([2, SRC], fp32, name="glT2", tag="psm2")
                for ct in range(4):
                    nc.tensor.matmul(glT2[:], wd_t[ct][:, 1:3], hT[ct][:],
                                     start=(ct == 0), stop=(ct == 3))
                egT2 = sml_pool.tile([2, SRC], fp32, name="egT2", tag="egT2", bufs=1)
                nc.scalar.activation(egT2[:], glT2[:], AF.Exp)
                rcp2 = psm_pool.tile([2, SRC], fp32, name="rcp2", tag="psm")
                nc.tensor.matmul(rcp2[:], ones12[:], rcp3[:], start=True, stop=True)
                g2rows = sml_pool.tile([2, SRC], bf16, name="g2rows", tag="g2rows", bufs=2)
                nc.vector.tensor_tensor(g2rows[:], egT2[:], rcp2[:], ALU.mult)

                # ---- parsel rows + cpT (sorted, padded): per chunk [cap, S] bf16
                psel = sml_pool.tile([2, NCH * cap], bf16, name="psel", tag="psel", bufs=1)
                nc.sync.dma_start(psel[:], parsel_d[b])
                cpT = []
                for c in range(NCH):
                    gp = psm_pool.tile([cap, SRC], fp32, name="gp", tag="psm2")
                    nc.tensor.matmul(gp[:], psel[:, c * cap:(c + 1) * cap],
                                     g2rows[:], start=True, stop=True)
                    alg = alg_pool.tile([cap, SRC], bf16, name="alg", tag="alg")
                    nc.sync.dma_start(alg[:], alignP_d[b, c])
                    cp = cpT_pool.tile([cap, SRC], bf16, name="cp", tag=f"cp{c}")
                    nc.vector.tensor_tensor(cp[:], alg[:], gp[:], ALU.mult)
                    cpT.append(cp)

                # ---- per s-tile: gates(gen), base matmuls, exp+Z, combine
                for st in range(4):
                    sl = slice(st * 128, (st + 1) * 128)
                    # gen gate, row-major
                    gl = psm_pool.tile([128, 3], fp32, name="gl", tag="psm")
                    for ct in range(4):
                        nc.tensor.matmul(gl[:], hT[ct][:, sl], wd_t[ct][:],
                                         start=(ct == 0), stop=(ct == 3))
                    eg = sml_pool.tile([128, 3], fp32, name="eg", tag="eg")
                    zgr = sml_pool.tile([128, 1], fp32, name="zgr", tag="zgr")
                    nc.scalar.activation(eg[:], gl[:], AF.Exp, accum_out=zgr[:])
                    rzg = sml_pool.tile([128, 1], fp32, name="rzg", tag="rzg")
                    nc.vector.reciprocal(rzg[:], zgr[:])
                    gen = sml_pool.tile([128, 1], fp32, name="gen", tag="gen")
                    nc.vector.tensor_tensor(gen[:], eg[:, 0:1], rzg[:], ALU.mult)

                    # base logits -> exp -> e (bf16, two half tiles) + Z accumulation
                    e_half = [e_pool.tile([128, V // 2], bf16, name="e_t", tag="e")
                              for _ in range(2)]
                    zparts = sml_pool.tile([128, NCH - 1], fp32, name="zparts", tag="zp")
                    for c in range(NCH - 1):
                        xps = px_pool.tile([128, 512], fp32, name="xps", tag="px", bufs=kn["px_bufs"])
                        for ct in range(4):
                            nc.tensor.matmul(
                                xps[:, :CH],
                                hT[ct][:, sl],
                                wg_t[ct][:, c * CH:(c + 1) * CH],
                                start=(ct == 0), stop=(ct == 3))
                        nc.scalar.activation(
                            e_half[c // 12][:, (c % 12) * CH:((c % 12) + 1) * CH],
                            xps[:, :CH], AF.Exp,
                            accum_out=zparts[:, c:c + 1])
                    zr = sml_pool.tile([128, 1], fp32, name="zr", tag="zr")
                    nc.vector.tensor_reduce(zr[:], zparts[:], mybir.AxisListType.X, ALU.add)
                    rz = sml_pool.tile([128, 1], fp32, name="rz", tag="rz")
                    nc.vector.reciprocal(rz[:], zr[:])
                    c1 = sml_pool.tile([128, 1], fp32, name="c1", tag="c1")
                    nc.vector.tensor_tensor(c1[:], gen[:], rz[:], ALU.mult)

                    # combine chunks: ll = log(c1*e + scat + eps)
                    for g in range(6):
                        t_stg = stg_pool.tile([128, 2000], fp32, name="t_stg", tag="t")
                        for h in range(4):
                            c = 4 * g + h
                            oh = oh_pool.tile([cap, CH], bf16, name="oh", tag="oh")
                            nc.sync.dma_start(oh[:], onehotP_d[b, c])
                            scat = psc_pool.tile([128, CH], fp32, name="scat", tag="sc")
                            nc.tensor.matmul(scat[:], cpT[c][:, sl], oh[:],
                                             start=True, stop=True)
                            nc.vector.scalar_tensor_tensor(
                                t_stg[:, h * CH:(h + 1) * CH],
                                e_half[c // 12][:, (c % 12) * CH:((c % 12) + 1) * CH],
                                c1[:], scat[:], ALU.mult, ALU.add)
                        ll_stg = stg_pool.tile([128, 2000], fp32, name="ll_stg", tag="lo")
                        nc.scalar.activation(ll_stg[:], t_stg[:], AF.Ln, bias=epsb[:])
                        nc.sync.dma_start(
                            ll_d[sl, b, g * 2000:(g + 1) * 2000], ll_stg[:])
                    # ext chunk: ll = log(scat + eps)
                    oh = oh_pool.tile([cap, CH], bf16, name="oh", tag="oh")
                    nc.sync.dma_start(oh[:], onehotP_d[b, NCH - 1])
                    scat = psc_pool.tile([128, CH], fp32, name="scat", tag="sc")
                    nc.tensor.matmul(scat[:], cpT[NCH - 1][:, sl], oh[:],
                                     start=True, stop=True)
                    ext_stg = stg_pool.tile([128, CH], fp32, name="ext_stg", tag="ex")
                    nc.scalar.activation(ext_stg[:], scat[:], AF.Ln, bias=epsb[:])
                    nc.sync.dma_start(ll_d[sl, b, V:TOT], ext_stg[:])

            if not kn["arc_early"]:
                emit_arc()

    _split_multi_waits(nc)
    return nc


_CACHE = {}


def _get_bass(cap):
    if cap not in _CACHE:
        _CACHE[cap] = _build_bass(cap)
    return _CACHE[cap]


def kernel(**inputs):
    in_maps, cap = _host_prep(inputs)
    nc = _get_bass(cap)
    res = run_bass_kernel_spmd(nc, in_maps, core_ids=list(range(NCORES)))
    ll = np.concatenate([res.results[i]["ll"] for i in range(NCORES)], axis=1)
    arcll = np.concatenate([res.results[i]["arcll"] for i in range(NCORES)], axis=1)
    return ll.astype(np.float32), arcll.astype(np.float32)


# revision 29
# speedup vs baseline: 99.4736x; 99.4736x over previous
"""ArcConceptDecoder (work=True inference path) as a distributed Bass kernel
on 8 TRN2 NeuronCores.

Sharding: batch (B=16) split as one b-pair per core (b = 2*core, 2*core+1).
The scatter_add is row-local per (src, b) and its indices (copy_seq) are known
at kernel-build time, so the scatter is realized as per-chunk compact one-hot
matmuls on TensorE with a structure that is uniform across cores (padded to a
global per-chunk row capacity CAP).

Per core, per b:
  hT[C,S]      = tanh(WtT.T @ coT)                  (bf16, TensorE+ACT)
  gates        = softmax(h @ Wd.T) row-major (gen gate) and transposed (2 rows)
  cpT[j,S]     = alignP (host-sorted/padded rows of align.T) * gate_perm
  x[S,1000]    = hT.T @ WgT chunks (PSUM), exp -> e (bf16) with accum -> Z
  scat[S,500]  = cpT_chunk.T @ onehot_chunk (PSUM)
  ll           = log((gen/Z) * e + scat + 1e-12)    (STT on DVE + log on ACT)
  ext chunk    = log(scat + 1e-12)
  arc_ll       = log(arc + 1e-12)
"""

import numpy as np
import ml_dtypes

import concourse.bass as bass
import concourse.mybir as mybir
import concourse.tile as tile
from concourse.bass_utils import run_bass_kernel_spmd

BF16 = ml_dtypes.bfloat16

SRC, B, SNT, E, C, V, EXT = 512, 16, 512, 512, 512, 12000, 500
TOT = V + EXT            # 12500
CH = 500                 # output chunk width (PSUM bank holds 512 f32)
NCH = TOT // CH          # 25 chunks; chunk 24 is the ext (scatter-only) region
GRP = 1000               # base-matmul group width (2 chunks per PSUM group)
NGRP = V // GRP          # 12 groups of the softmax region
NB = 2                   # batch entries per core
NCORES = 8
EPS = 1e-12


# ---------------------------------------------------------------- host tables
def _host_tables(copy_seq):
    """Sorted scatter tables per b + global row capacity CAP per chunk."""
    copy_seq = np.asarray(copy_seq)[:SNT]     # [SNT, B, 2]
    per_b = []
    max_cnt = 0
    for b in range(B):
        tgt = np.concatenate([copy_seq[:, b, 0], copy_seq[:, b, 1]]).astype(np.int64)
        par = np.concatenate([np.zeros(SNT, np.int64), np.ones(SNT, np.int64)])
        t_idx = np.concatenate([np.arange(SNT), np.arange(SNT)])
        order = np.argsort(tgt, kind="stable")
        tgt_s, par_s, t_s = tgt[order], par[order], t_idx[order]
        chunk = tgt_s // CH
        pos = (tgt_s % CH).astype(np.int64)
        starts = np.searchsorted(chunk, np.arange(NCH))
        ends = np.searchsorted(chunk, np.arange(NCH), side="right")
        max_cnt = max(max_cnt, int((ends - starts).max()))
        per_b.append((t_s, par_s, pos, starts, ends))
    cap = ((max_cnt + 31) // 32) * 32
    return per_b, cap


def _host_prep(inputs):
    aw = np.asarray(inputs["alignment_weight"], np.float32)   # [S, B, SNT]
    arc = np.asarray(inputs["arc_weight"], np.float32)        # [S, B, S]
    co = np.asarray(inputs["concept_outs"], np.float32)       # [S, B, E]
    Wt = np.asarray(inputs["W_transfer"], np.float32)         # [C, E]
    Wg = np.asarray(inputs["W_gen"], np.float32)              # [V, C]
    Wd = np.asarray(inputs["W_div"], np.float32)              # [3, C]
    for nm in ("b_transfer", "b_gen", "b_div"):
        assert not np.asarray(inputs[nm]).any(), f"{nm} must be zero"

    per_b, cap = _host_tables(inputs["copy_seq"])

    WtT = np.ascontiguousarray(Wt.T).astype(BF16).reshape(4, 128, C)
    WdT = np.ascontiguousarray(Wd.T).astype(BF16).reshape(4, 128, 3)
    WgT = np.ascontiguousarray(Wg.T).astype(ml_dtypes.float8_e4m3).reshape(4, 128, V)

    in_maps = []
    for core in range(NCORES):
        bs = [NB * core + i for i in range(NB)]
        coT = np.stack(
            [np.ascontiguousarray(co[:, b, :].T).astype(BF16).reshape(4, 128, SRC)
             for b in bs])                                     # [NB,4,128,S]
        alignP = np.zeros((NB, NCH, cap, SRC), BF16)
        onehotP = np.zeros((NB, NCH, cap, CH), BF16)
        parsel = np.zeros((NB, 2, NCH * cap), BF16)
        for i, b in enumerate(bs):
            t_s, par_s, pos, starts, ends = per_b[b]
            awT = aw[:, b, :].T.astype(BF16)                   # [SNT, S]
            for c in range(NCH):
                s0, s1 = int(starts[c]), int(ends[c])
                n = s1 - s0
                if n == 0:
                    continue
                alignP[i, c, :n] = awT[t_s[s0:s1]]
                onehotP[i, c, np.arange(n), pos[s0:s1]] = BF16(1.0)
                # gate row selector: row0 = map gate (par=1), row1 = copy (par=0)
                sel_row = np.where(par_s[s0:s1] == 0, 1, 0)
                parsel[i, sel_row, c * cap + np.arange(n)] = BF16(1.0)
        arc_s = np.ascontiguousarray(arc[:, bs, :])            # [S, NB, S]
        in_maps.append(dict(
            coT=coT, WtT=WtT, WdT=WdT, WgT=WgT,
            alignP=alignP, onehotP=onehotP, parsel=parsel, arc=arc_s,
        ))
    return in_maps, cap


# ---------------------------------------------------------------- bass kernel
def _split_multi_waits(nc):
    """This walrus build only accepts one sync-wait per instruction: split
    multi-wait instructions by hoisting extra waits onto same-engine drains."""
    n = 0
    for f in nc.m.functions:
        for blk in f.blocks:
            new_insts = []
            for inst in blk.instructions:
                si = inst.sync_info
                if si is not None and si.on_wait and len(si.on_wait) > 1:
                    waits = list(si.on_wait)
                    for w in waits[:-1]:
                        nop = mybir.InstDrain(name=f"WSPLIT-{n}", ins=[], outs=[])
                        nop.engine = inst.engine
                        nop.sync_info = mybir.SyncInfo(on_wait=[w], on_update=[])
                        new_insts.append(nop)
                        n += 1
                    inst.sync_info = mybir.SyncInfo(
                        on_wait=[waits[-1]], on_update=list(si.on_update))
                new_insts.append(inst)
            blk.instructions = new_insts
    return n


def _build_bass(cap, knobs=None):
    kn = dict(arc_early=False, cpt_bufs=1, e_bufs=3, px_bufs=4, psc_bufs=2,
              oh_bufs=8, stg_bufs=3, alg_bufs=4, hT_bufs=2)
    if knobs:
        kn.update(knobs)
    fp32 = mybir.dt.float32
    bf16 = mybir.dt.bfloat16
    AF = mybir.ActivationFunctionType
    ALU = mybir.AluOpType
    nc = bass.Bass()

    coT_d = nc.declare_dram_parameter("coT", [NB, 4, 128, SRC], bf16, isOutput=False)
    WtT_d = nc.declare_dram_parameter("WtT", [4, 128, C], bf16, isOutput=False)
    WdT_d = nc.declare_dram_parameter("WdT", [4, 128, 3], bf16, isOutput=False)
    WgT_d = nc.declare_dram_parameter("WgT", [4, 128, V], mybir.dt.float8e4, isOutput=False)
    alignP_d = nc.declare_dram_parameter("alignP", [NB, NCH, cap, SRC], bf16, isOutput=False)
    onehotP_d = nc.declare_dram_parameter("onehotP", [NB, NCH, cap, CH], bf16, isOutput=False)
    parsel_d = nc.declare_dram_parameter("parsel", [NB, 2, NCH * cap], bf16, isOutput=False)
    arc_d = nc.declare_dram_parameter("arc", [SRC, NB, SRC], fp32, isOutput=False)
    ll_d = nc.declare_dram_parameter("ll", [SRC, NB, TOT], fp32, isOutput=True)
    arcll_d = nc.declare_dram_parameter("arcll", [SRC, NB, SRC], fp32, isOutput=True)

    with tile.TileContext(nc) as tc:
        with (
            tc.tile_pool(name="wg", bufs=1) as wg_pool,
            tc.tile_pool(name="wsm", bufs=1) as wsm_pool,
            tc.tile_pool(name="hT", bufs=kn["hT_bufs"]) as hT_pool,
            tc.tile_pool(name="cpT", bufs=kn["cpt_bufs"]) as cpT_pool,
            tc.tile_pool(name="oh", bufs=kn["oh_bufs"]) as oh_pool,
            tc.tile_pool(name="alg", bufs=kn["alg_bufs"]) as alg_pool,
            tc.tile_pool(name="e", bufs=kn["e_bufs"]) as e_pool,
            tc.tile_pool(name="stg", bufs=kn["stg_bufs"]) as stg_pool,
            tc.tile_pool(name="arcp", bufs=2) as arc_pool,
            tc.tile_pool(name="sml", bufs=2) as sml_pool,
            tc.tile_pool(name="px", bufs=kn["px_bufs"], space="PSUM") as px_pool,
            tc.tile_pool(name="psc", bufs=kn["psc_bufs"], space="PSUM") as psc_pool,
            tc.tile_pool(name="psm", bufs=1, space="PSUM") as psm_pool,
        ):
            epsb = wsm_pool.tile([128, 1], fp32, name="epsb", tag="epsb")
            nc.vector.memset(epsb[:], EPS)

            def emit_arc():
                for b in range(NB):
                    for st in range(4):
                        a_in = arc_pool.tile([128, SRC], fp32, name="a_in", tag="a_in")
                        nc.sync.dma_start(a_in[:], arc_d[st * 128:(st + 1) * 128, b, :])
                        a_out = arc_pool.tile([128, SRC], fp32, name="a_out", tag="a_out")
                        nc.scalar.activation(a_out[:], a_in[:], AF.Ln, bias=epsb[:])
                        nc.sync.dma_start(arcll_d[st * 128:(st + 1) * 128, b, :], a_out[:])

            if kn["arc_early"]:
                emit_arc()

            # ---- resident weights
            wg_t = []
            for kt in range(4):
                w = wg_pool.tile([128, V], mybir.dt.float8e4, name=f"wg{kt}", tag=f"wg{kt}")
                nc.sync.dma_start(w[:], WgT_d[kt])
                wg_t.append(w)
            wt_t, wd_t = [], []
            for kt in range(4):
                w1 = wsm_pool.tile([128, C], bf16, name=f"wt{kt}", tag=f"wt{kt}")
                nc.sync.dma_start(w1[:], WtT_d[kt])
                wt_t.append(w1)
                w2 = wsm_pool.tile([128, 3], bf16, name=f"wd{kt}", tag=f"wd{kt}")
                nc.sync.dma_start(w2[:], WdT_d[kt])
                wd_t.append(w2)
            ones3 = wsm_pool.tile([3, 1], fp32, name="ones3", tag="ones3")
            nc.vector.memset(ones3[:], 1.0)
            ones12 = wsm_pool.tile([1, 2], fp32, name="ones12", tag="ones12")
            nc.vector.memset(ones12[:], 1.0)

            for b in range(NB):
                # ---- hT = tanh(WtT.T @ coT): [C(4x128), S] bf16
                co_t = []
                for et in range(4):
                    cot = sml_pool.tile([128, SRC], bf16, name="cot", tag="cot", bufs=4)
                    nc.sync.dma_start(cot[:], coT_d[b, et])
                    co_t.append(cot)
                hT = []
                for ct in range(4):
                    hps = px_pool.tile([128, 512], fp32, name="hps", tag="px", bufs=kn["px_bufs"])
                    for et in range(4):
                        nc.tensor.matmul(
                            hps[:],
                            wt_t[et][:, ct * 128:(ct + 1) * 128],
                            co_t[et][:],
                            start=(et == 0), stop=(et == 3))
                    ht = hT_pool.tile([128, SRC], bf16, name="ht", tag=f"ht{ct}")
                    nc.scalar.activation(ht[:], hps[:], AF.Tanh)
                    hT.append(ht)

                # ---- transposed gate rows g2rows [2, S]: row0=map, row1=copy
                glT = psm_pool.tile([3, SRC], fp32, name="glT", tag="psm")
                for ct in range(4):
                    nc.tensor.matmul(glT[:], wd_t[ct][:], hT[ct][:],
                                     start=(ct == 0), stop=(ct == 3))
                egT = sml_pool.tile([3, SRC], fp32, name="egT", tag="egT", bufs=1)
                nc.scalar.activation(egT[:], glT[:], AF.Exp)
                zg3 = psm_pool.tile([1, SRC], fp32, name="zg3", tag="psm")
                nc.tensor.matmul(zg3[:], ones3[:], egT[:], start=True, stop=True)
                rcp3 = sml_pool.tile([1, SRC], fp32, name="rcp3", tag="rcp3", bufs=1)
                nc.vector.reciprocal(rcp3[:], zg3[:])
                # [2, S] psum of the (map, copy) logit rows -> exp -> * 1/Z
                glT2 = psm_pool.tile([2, SRC], fp32, name="glT2", tag="psm2")
                for ct in range(4):
                    nc.tensor.matmul(glT2[:], wd_t[ct][:, 1:3], hT[ct][:],
                                     start=(ct == 0), stop=(ct == 3))
                egT2 = sml_pool.tile([2, SRC], fp32, name="egT2", tag="egT2", bufs=1)
                nc.scalar.activation(egT2[:], glT2[:], AF.Exp)
                rcp2 = psm_pool.tile([2, SRC], fp32, name="rcp2", tag="psm")
                nc.tensor.matmul(rcp2[:], ones12[:], rcp3[:], start=True, stop=True)
                g2rows = sml_pool.tile([2, SRC], bf16, name="g2rows", tag="g2rows", bufs=2)
                nc.vector.tensor_tensor(g2rows[:], egT2[:], rcp2[:], ALU.mult)

                # ---- parsel rows + cpT (sorted, padded): per chunk [cap, S] bf16
                psel = sml_pool.tile([2, NCH * cap], bf16, name="psel", tag="psel", bufs=1)
                nc.sync.dma_start(psel[:], parsel_d[b])
                cpT = []
                for c in range(NCH):
                    gp = psm_pool.tile([cap, SRC], fp32, name="gp", tag="psm2")
                    nc.tensor.matmul(gp[:], psel[:, c * cap:(c + 1) * cap],
                                     g2rows[:], start=True, stop=True)
                    alg = alg_pool.tile([cap, SRC], bf16, name="alg", tag="alg")
                    nc.sync.dma_start(alg[:], alignP_d[b, c])
                    cp = cpT_pool.tile([cap, SRC], bf16, name="cp", tag=f"cp{c}")
                    nc.vector.tensor_tensor(cp[:], alg[:], gp[:], ALU.mult)
                    cpT.append(cp)

                # ---- per s-tile: gates(gen), base matmuls, exp+Z, combine
                for st in range(4):
                    sl = slice(st * 128, (st + 1) * 128)
                    # gen gate, row-major
                    gl = psm_pool.tile([128, 3], fp32, name="gl", tag="psm")
                    for ct in range(4):
                        nc.tensor.matmul(gl[:], hT[ct][:, sl], wd_t[ct][:],
                                         start=(ct == 0), stop=(ct == 3))
                    eg = sml_pool.tile([128, 3], fp32, name="eg", tag="eg")
                    zgr = sml_pool.tile([128, 1], fp32, name="zgr", tag="zgr")
                    nc.scalar.activation(eg[:], gl[:], AF.Exp, accum_out=zgr[:])
                    rzg = sml_pool.tile([128, 1], fp32, name="rzg", tag="rzg")
                    nc.vector.reciprocal(rzg[:], zgr[:])
                    gen = sml_pool.tile([128, 1], fp32, name="gen", tag="gen")
                    nc.vector.tensor_tensor(gen[:], eg[:, 0:1], rzg[:], ALU.mult)

                    # base logits -> exp -> e (bf16, two half tiles) + Z accumulation
                    e_half = [e_pool.tile([128, V // 2], bf16, name="e_t", tag="e")
                              for _ in range(2)]
                    zparts = sml_pool.tile([128, NCH - 1], fp32, name="zparts", tag="zp")
                    for c in range(NCH - 1):
                        xps = px_pool.tile([128, 512], fp32, name="xps", tag="px", bufs=kn["px_bufs"])
                        for ct in range(4):
                            nc.tensor.matmul(
                                xps[:, :CH],
                                hT[ct][:, sl],
                                wg_t[ct][:, c * CH:(c + 1) * CH],
                                start=(ct == 0), stop=(ct == 3))
                        nc.scalar.activation(
                            e_half[c // 12][:, (c % 12) * CH:((c % 12) + 1) * CH],
                            xps[:, :CH], AF.Exp,
                            accum_out=zparts[:, c:c + 1])
                    zr = sml_pool.tile([128, 1], fp32, name="zr", tag="zr")
                    nc.vector.tensor_reduce(zr[:], zparts[:], mybir.AxisListType.X, ALU.add)
                    rz = sml_pool.tile([128, 1], fp32, name="rz", tag="rz")
                    nc.vector.reciprocal(rz[:], zr[:])
                    c1 = sml_pool.tile([128, 1], fp32, name="c1", tag="c1")
                    nc.vector.tensor_tensor(c1[:], gen[:], rz[:], ALU.mult)

                    # combine chunks: ll = log(c1*e + scat + eps)
                    for g in range(6):
                        t_stg = stg_pool.tile([128, 2000], fp32, name="t_stg", tag="t")
                        for h in range(4):
                            c = 4 * g + h
                            oh = oh_pool.tile([cap, CH], bf16, name="oh", tag="oh")
                            nc.sync.dma_start(oh[:], onehotP_d[b, c])
                            scat = psc_pool.tile([128, CH], fp32, name="scat", tag="sc")
                            nc.tensor.matmul(scat[:], cpT[c][:, sl], oh[:],
                                             start=True, stop=True)
                            nc.vector.scalar_tensor_tensor(
                                t_stg[:, h * CH:(h + 1) * CH],
                                e_half[c // 12][:, (c % 12) * CH:((c % 12) + 1) * CH],
                                c1[:], scat[:], ALU.mult, ALU.add)
                        ll_stg = stg_pool.tile([128, 2000], fp32, name="ll_stg", tag="lo")
                        nc.scalar.activation(ll_stg[:], t_stg[:], AF.Ln, bias=epsb[:])
                        nc.sync.dma_start(
                            ll_d[sl, b, g * 2000:(g + 1) * 2000], ll_stg[:])
                    # ext chunk: ll = log(scat + eps)
                    oh = oh_pool.tile([cap, CH], bf16, name="oh", tag="oh")
                    nc.sync.dma_start(oh[:], onehotP_d[b, NCH - 1])
                    scat = psc_pool.tile([128, CH], fp32, name="scat", tag="sc")
                    nc.tensor.matmul(scat[:], cpT[NCH - 1][:, sl], oh[:],
                                     start=True, stop=True)
                    ext_stg = stg_pool.tile([128, CH], fp32, name="ext_stg", tag="ex")
                    nc.scalar.activation(ext_stg[:], scat[:], AF.Ln, bias=epsb[:])
                    nc.sync.dma_start(ll_d[sl, b, V:TOT], ext_stg[:])

            if not kn["arc_early"]:
                emit_arc()

    _split_multi_waits(nc)
    return nc


_CACHE = {}


def _get_bass(cap):
    if cap not in _CACHE:
        _CACHE[cap] = _build_bass(cap)
    return _CACHE[cap]


def kernel(**inputs):
    in_maps, cap = _host_prep(inputs)
    nc = _get_bass(cap)
    res = run_bass_kernel_spmd(nc, in_maps, core_ids=list(range(NCORES)))
    ll = np.concatenate([res.results[i]["ll"] for i in range(NCORES)], axis=1)
    arcll = np.concatenate([res.results[i]["arcll"] for i in range(NCORES)], axis=1)
    return ll.astype(np.float32), arcll.astype(np.float32)


# revision 34
# speedup vs baseline: 114.5210x; 1.1513x over previous
"""ArcConceptDecoder (work=True inference path) as a distributed Bass kernel
on 8 TRN2 NeuronCores.

Sharding: batch (B=16) split as one b-pair per core (b = 2*core, 2*core+1).
The scatter_add is row-local per (src, b) and its indices (copy_seq) are known
at kernel-build time, so the scatter is realized as per-chunk compact one-hot
matmuls on TensorE with a structure that is uniform across cores (padded to a
global per-chunk row capacity CAP).

Per core, per b:
  hT[C,S]      = tanh(WtT.T @ coT)                  (bf16, TensorE+ACT)
  gates        = softmax(h @ Wd.T) row-major (gen gate) and transposed (2 rows)
  cpT[j,S]     = alignP (host-sorted/padded rows of align.T) * gate_perm
  x[S,1000]    = hT.T @ WgT chunks (PSUM), exp -> e (bf16) with accum -> Z
  scat[S,500]  = cpT_chunk.T @ onehot_chunk (PSUM)
  ll           = log((gen/Z) * e + scat + 1e-12)    (STT on DVE + log on ACT)
  ext chunk    = log(scat + 1e-12)
  arc_ll       = log(arc + 1e-12)
"""

import numpy as np
import ml_dtypes

import concourse.bass as bass
import concourse.mybir as mybir
import concourse.tile as tile
from concourse.bass_utils import run_bass_kernel_spmd

BF16 = ml_dtypes.bfloat16

SRC, B, SNT, E, C, V, EXT = 512, 16, 512, 512, 512, 12000, 500
TOT = V + EXT            # 12500
CH = 500                 # output chunk width (PSUM bank holds 512 f32)
NCH = TOT // CH          # 25 chunks; chunk 24 is the ext (scatter-only) region
GRP = 1000               # base-matmul group width (2 chunks per PSUM group)
NGRP = V // GRP          # 12 groups of the softmax region
NB = 2                   # batch entries per core
NCORES = 8
EPS = 1e-12


# ---------------------------------------------------------------- host tables
def _host_tables(copy_seq):
    """Sorted scatter tables per b + global row capacity CAP per chunk."""
    copy_seq = np.asarray(copy_seq)[:SNT]     # [SNT, B, 2]
    per_b = []
    max_cnt = 0
    for b in range(B):
        tgt = np.concatenate([copy_seq[:, b, 0], copy_seq[:, b, 1]]).astype(np.int64)
        par = np.concatenate([np.zeros(SNT, np.int64), np.ones(SNT, np.int64)])
        t_idx = np.concatenate([np.arange(SNT), np.arange(SNT)])
        order = np.argsort(tgt, kind="stable")
        tgt_s, par_s, t_s = tgt[order], par[order], t_idx[order]
        chunk = tgt_s // CH
        pos = (tgt_s % CH).astype(np.int64)
        starts = np.searchsorted(chunk, np.arange(NCH))
        ends = np.searchsorted(chunk, np.arange(NCH), side="right")
        max_cnt = max(max_cnt, int((ends - starts).max()))
        per_b.append((t_s, par_s, pos, starts, ends))
    cap = ((max_cnt + 31) // 32) * 32
    return per_b, cap


def _host_prep(inputs):
    aw = np.asarray(inputs["alignment_weight"], np.float32)   # [S, B, SNT]
    arc = np.asarray(inputs["arc_weight"], np.float32)        # [S, B, S]
    co = np.asarray(inputs["concept_outs"], np.float32)       # [S, B, E]
    Wt = np.asarray(inputs["W_transfer"], np.float32)         # [C, E]
    Wg = np.asarray(inputs["W_gen"], np.float32)              # [V, C]
    Wd = np.asarray(inputs["W_div"], np.float32)              # [3, C]
    for nm in ("b_transfer", "b_gen", "b_div"):
        assert not np.asarray(inputs[nm]).any(), f"{nm} must be zero"

    per_b, cap = _host_tables(inputs["copy_seq"])

    WtT = np.ascontiguousarray(Wt.T).astype(BF16).reshape(4, 128, C)
    WdT = np.ascontiguousarray(Wd.T).astype(BF16).reshape(4, 128, 3)
    WgT = np.ascontiguousarray(Wg.T).astype(ml_dtypes.float8_e4m3).reshape(4, 128, V)

    in_maps = []
    for core in range(NCORES):
        bs = [NB * core + i for i in range(NB)]
        coT = np.stack(
            [np.ascontiguousarray(co[:, b, :].T).astype(BF16).reshape(4, 128, SRC)
             for b in bs])                                     # [NB,4,128,S]
        alignP = np.zeros((NB, NCH, cap, SRC), BF16)
        onehotP = np.zeros((NB, cap, NCH * CH), BF16)
        parsel = np.zeros((NB, 2, NCH * cap), BF16)
        for i, b in enumerate(bs):
            t_s, par_s, pos, starts, ends = per_b[b]
            awT = aw[:, b, :].T.astype(BF16)                   # [SNT, S]
            for c in range(NCH):
                s0, s1 = int(starts[c]), int(ends[c])
                n = s1 - s0
                if n == 0:
                    continue
                alignP[i, c, :n] = awT[t_s[s0:s1]]
                onehotP[i, np.arange(n), c * CH + pos[s0:s1]] = BF16(1.0)
                # gate row selector: row0 = map gate (par=1), row1 = copy (par=0)
                sel_row = np.where(par_s[s0:s1] == 0, 1, 0)
                parsel[i, sel_row, c * cap + np.arange(n)] = BF16(1.0)
        arc_s = np.ascontiguousarray(arc[:, bs, :])            # [S, NB, S]
        in_maps.append(dict(
            coT=coT, WtT=WtT, WdT=WdT, WgT=WgT,
            alignP=alignP, onehotP=onehotP, parsel=parsel, arc=arc_s,
        ))
    return in_maps, cap


# ---------------------------------------------------------------- bass kernel
def _split_multi_waits(nc):
    """This walrus build only accepts one sync-wait per instruction: split
    multi-wait instructions by hoisting extra waits onto same-engine drains."""
    n = 0
    for f in nc.m.functions:
        for blk in f.blocks:
            new_insts = []
            for inst in blk.instructions:
                si = inst.sync_info
                if si is not None and si.on_wait and len(si.on_wait) > 1:
                    waits = list(si.on_wait)
                    for w in waits[:-1]:
                        nop = mybir.InstDrain(name=f"WSPLIT-{n}", ins=[], outs=[])
                        nop.engine = inst.engine
                        nop.sync_info = mybir.SyncInfo(on_wait=[w], on_update=[])
                        new_insts.append(nop)
                        n += 1
                    inst.sync_info = mybir.SyncInfo(
                        on_wait=[waits[-1]], on_update=list(si.on_update))
                new_insts.append(inst)
            blk.instructions = new_insts
    return n


def _build_bass(cap, knobs=None):
    kn = dict(arc_early=False, cpt_bufs=1, e_bufs=3, px_bufs=4, psc_bufs=2,
              oh_bufs=8, stg_bufs=3, alg_bufs=4, hT_bufs=2)
    if knobs:
        kn.update(knobs)
    fp32 = mybir.dt.float32
    bf16 = mybir.dt.bfloat16
    AF = mybir.ActivationFunctionType
    ALU = mybir.AluOpType
    nc = bass.Bass()

    coT_d = nc.declare_dram_parameter("coT", [NB, 4, 128, SRC], bf16, isOutput=False)
    WtT_d = nc.declare_dram_parameter("WtT", [4, 128, C], bf16, isOutput=False)
    WdT_d = nc.declare_dram_parameter("WdT", [4, 128, 3], bf16, isOutput=False)
    WgT_d = nc.declare_dram_parameter("WgT", [4, 128, V], mybir.dt.float8e4, isOutput=False)
    alignP_d = nc.declare_dram_parameter("alignP", [NB, NCH, cap, SRC], bf16, isOutput=False)
    onehotP_d = nc.declare_dram_parameter("onehotP", [NB, cap, NCH * CH], bf16, isOutput=False)
    parsel_d = nc.declare_dram_parameter("parsel", [NB, 2, NCH * cap], bf16, isOutput=False)
    arc_d = nc.declare_dram_parameter("arc", [SRC, NB, SRC], fp32, isOutput=False)
    ll_d = nc.declare_dram_parameter("ll", [SRC, NB, TOT], fp32, isOutput=True)
    arcll_d = nc.declare_dram_parameter("arcll", [SRC, NB, SRC], fp32, isOutput=True)

    with tile.TileContext(nc) as tc:
        with (
            tc.tile_pool(name="wg", bufs=1) as wg_pool,
            tc.tile_pool(name="wsm", bufs=1) as wsm_pool,
            tc.tile_pool(name="hT", bufs=kn["hT_bufs"]) as hT_pool,
            tc.tile_pool(name="cpT", bufs=kn["cpt_bufs"]) as cpT_pool,
            tc.tile_pool(name="oh", bufs=kn["oh_bufs"]) as oh_pool,
            tc.tile_pool(name="alg", bufs=kn["alg_bufs"]) as alg_pool,
            tc.tile_pool(name="e", bufs=kn["e_bufs"]) as e_pool,
            tc.tile_pool(name="stg", bufs=kn["stg_bufs"]) as stg_pool,
            tc.tile_pool(name="arcp", bufs=2) as arc_pool,
            tc.tile_pool(name="sml", bufs=2) as sml_pool,
            tc.tile_pool(name="px", bufs=kn["px_bufs"], space="PSUM") as px_pool,
            tc.tile_pool(name="psc", bufs=kn["psc_bufs"], space="PSUM") as psc_pool,
            tc.tile_pool(name="psm", bufs=1, space="PSUM") as psm_pool,
        ):
            epsb = wsm_pool.tile([128, 1], fp32, name="epsb", tag="epsb")
            nc.vector.memset(epsb[:], EPS)

            def emit_arc():
                for b in range(NB):
                    for st in range(4):
                        a_in = arc_pool.tile([128, SRC], fp32, name="a_in", tag="a_in")
                        nc.sync.dma_start(a_in[:], arc_d[st * 128:(st + 1) * 128, b, :])
                        a_out = arc_pool.tile([128, SRC], fp32, name="a_out", tag="a_out")
                        nc.scalar.activation(a_out[:], a_in[:], AF.Ln, bias=epsb[:])
                        nc.sync.dma_start(arcll_d[st * 128:(st + 1) * 128, b, :], a_out[:])

            if kn["arc_early"]:
                emit_arc()

            # ---- resident weights
            wg_t = []
            for kt in range(4):
                w = wg_pool.tile([128, V], mybir.dt.float8e4, name=f"wg{kt}", tag=f"wg{kt}")
                nc.sync.dma_start(w[:], WgT_d[kt])
                wg_t.append(w)
            wt_t, wd_t = [], []
            for kt in range(4):
                w1 = wsm_pool.tile([128, C], bf16, name=f"wt{kt}", tag=f"wt{kt}")
                nc.sync.dma_start(w1[:], WtT_d[kt])
                wt_t.append(w1)
                w2 = wsm_pool.tile([128, 3], bf16, name=f"wd{kt}", tag=f"wd{kt}")
                nc.sync.dma_start(w2[:], WdT_d[kt])
                wd_t.append(w2)
            ones3 = wsm_pool.tile([3, 1], fp32, name="ones3", tag="ones3")
            nc.vector.memset(ones3[:], 1.0)
            ones12 = wsm_pool.tile([1, 2], fp32, name="ones12", tag="ones12")
            nc.vector.memset(ones12[:], 1.0)

            for b in range(NB):
                # ---- hT = tanh(WtT.T @ coT): [C(4x128), S] bf16
                co_t = []
                for et in range(4):
                    cot = sml_pool.tile([128, SRC], bf16, name="cot", tag="cot", bufs=4)
                    nc.sync.dma_start(cot[:], coT_d[b, et])
                    co_t.append(cot)
                hT = []
                for ct in range(4):
                    hps = px_pool.tile([128, 512], fp32, name="hps", tag="px", bufs=kn["px_bufs"])
                    for et in range(4):
                        nc.tensor.matmul(
                            hps[:],
                            wt_t[et][:, ct * 128:(ct + 1) * 128],
                            co_t[et][:],
                            start=(et == 0), stop=(et == 3))
                    ht = hT_pool.tile([128, SRC], bf16, name="ht", tag=f"ht{ct}")
                    nc.scalar.activation(ht[:], hps[:], AF.Tanh)
                    hT.append(ht)

                # ---- transposed gate rows g2rows [2, S]: row0=map, row1=copy
                glT = psm_pool.tile([3, SRC], fp32, name="glT", tag="psm")
                for ct in range(4):
                    nc.tensor.matmul(glT[:], wd_t[ct][:], hT[ct][:],
                                     start=(ct == 0), stop=(ct == 3))
                egT = sml_pool.tile([3, SRC], fp32, name="egT", tag="egT", bufs=1)
                nc.scalar.activation(egT[:], glT[:], AF.Exp)
                zg3 = psm_pool.tile([1, SRC], fp32, name="zg3", tag="psm")
                nc.tensor.matmul(zg3[:], ones3[:], egT[:], start=True, stop=True)
                rcp3 = sml_pool.tile([1, SRC], fp32, name="rcp3", tag="rcp3", bufs=1)
                nc.vector.reciprocal(rcp3[:], zg3[:])
                # [2, S] psum of the (map, copy) logit rows -> exp -> * 1/Z
                glT2 = psm_pool.tile([2, SRC], fp32, name="glT2", tag="psm2")
                for ct in range(4):
                    nc.tensor.matmul(glT2[:], wd_t[ct][:, 1:3], hT[ct][:],
                                     start=(ct == 0), stop=(ct == 3))
                egT2 = sml_pool.tile([2, SRC], fp32, name="egT2", tag="egT2", bufs=1)
                nc.scalar.activation(egT2[:], glT2[:], AF.Exp)
                rcp2 = psm_pool.tile([2, SRC], fp32, name="rcp2", tag="psm")
                nc.tensor.matmul(rcp2[:], ones12[:], rcp3[:], start=True, stop=True)
                g2rows = sml_pool.tile([2, SRC], bf16, name="g2rows", tag="g2rows", bufs=2)
                nc.vector.tensor_tensor(g2rows[:], egT2[:], rcp2[:], ALU.mult)

                # ---- parsel rows + cpT (sorted, padded): per chunk [cap, S] bf16
                psel = sml_pool.tile([2, NCH * cap], bf16, name="psel", tag="psel", bufs=1)
                nc.sync.dma_start(psel[:], parsel_d[b])
                ohbig = oh_pool.tile([cap, NCH * CH], bf16, name="ohbig", tag="ohbig", bufs=1)
                nc.sync.dma_start(ohbig[:], onehotP_d[b])
                cpT = []
                for c in range(NCH):
                    gp = psm_pool.tile([cap, SRC], fp32, name="gp", tag="psm2")
                    nc.tensor.matmul(gp[:], psel[:, c * cap:(c + 1) * cap],
                                     g2rows[:], start=True, stop=True)
                    alg = alg_pool.tile([cap, SRC], bf16, name="alg", tag="alg")
                    nc.sync.dma_start(alg[:], alignP_d[b, c])
                    cp = cpT_pool.tile([cap, SRC], bf16, name="cp", tag=f"cp{c}")
                    nc.vector.tensor_tensor(cp[:], alg[:], gp[:], ALU.mult)
                    cpT.append(cp)

                # ---- per s-tile: gates(gen), base matmuls, exp+Z, combine
                for st in range(4):
                    sl = slice(st * 128, (st + 1) * 128)
                    # gen gate, row-major
                    gl = psm_pool.tile([128, 3], fp32, name="gl", tag="psm")
                    for ct in range(4):
                        nc.tensor.matmul(gl[:], hT[ct][:, sl], wd_t[ct][:],
                                         start=(ct == 0), stop=(ct == 3))
                    eg = sml_pool.tile([128, 3], fp32, name="eg", tag="eg")
                    zgr = sml_pool.tile([128, 1], fp32, name="zgr", tag="zgr")
                    nc.scalar.activation(eg[:], gl[:], AF.Exp, accum_out=zgr[:])
                    rzg = sml_pool.tile([128, 1], fp32, name="rzg", tag="rzg")
                    nc.vector.reciprocal(rzg[:], zgr[:])
                    gen = sml_pool.tile([128, 1], fp32, name="gen", tag="gen")
                    nc.vector.tensor_tensor(gen[:], eg[:, 0:1], rzg[:], ALU.mult)

                    # base logits -> exp -> e (bf16, two half tiles) + Z accumulation
                    e_half = [e_pool.tile([128, V // 2], bf16, name="e_t", tag="e")
                              for _ in range(2)]
                    zparts = sml_pool.tile([128, NCH - 1], fp32, name="zparts", tag="zp")
                    for c in range(NCH - 1):
                        xps = px_pool.tile([128, 512], fp32, name="xps", tag="px", bufs=kn["px_bufs"])
                        for ct in range(4):
                            nc.tensor.matmul(
                                xps[:, :CH],
                                hT[ct][:, sl],
                                wg_t[ct][:, c * CH:(c + 1) * CH],
                                start=(ct == 0), stop=(ct == 3))
                        nc.scalar.activation(
                            e_half[c // 12][:, (c % 12) * CH:((c % 12) + 1) * CH],
                            xps[:, :CH], AF.Exp,
                            accum_out=zparts[:, c:c + 1])
                    zr = sml_pool.tile([128, 1], fp32, name="zr", tag="zr")
                    nc.vector.tensor_reduce(zr[:], zparts[:], mybir.AxisListType.X, ALU.add)
                    rz = sml_pool.tile([128, 1], fp32, name="rz", tag="rz")
                    nc.vector.reciprocal(rz[:], zr[:])
                    c1 = sml_pool.tile([128, 1], fp32, name="c1", tag="c1")
                    nc.vector.tensor_tensor(c1[:], gen[:], rz[:], ALU.mult)

                    # combine chunks: ll = log(c1*e + scat + eps)
                    for g in range(6):
                        t_stg = stg_pool.tile([128, 2000], fp32, name="t_stg", tag="t")
                        for h in range(4):
                            c = 4 * g + h
                            scat = psc_pool.tile([128, CH], fp32, name="scat", tag="sc")
                            nc.tensor.matmul(scat[:], cpT[c][:, sl],
                                             ohbig[:, c * CH:(c + 1) * CH],
                                             start=True, stop=True)
                            nc.vector.scalar_tensor_tensor(
                                t_stg[:, h * CH:(h + 1) * CH],
                                e_half[c // 12][:, (c % 12) * CH:((c % 12) + 1) * CH],
                                c1[:], scat[:], ALU.mult, ALU.add)
                        nc.scalar.activation(t_stg[:], t_stg[:], AF.Ln, bias=epsb[:])
                        nc.sync.dma_start(
                            ll_d[sl, b, g * 2000:(g + 1) * 2000], t_stg[:])
                    # ext chunk: ll = log(scat + eps)
                    scat = psc_pool.tile([128, CH], fp32, name="scat", tag="sc")
                    nc.tensor.matmul(scat[:], cpT[NCH - 1][:, sl],
                                     ohbig[:, (NCH - 1) * CH:NCH * CH],
                                     start=True, stop=True)
                    ext_stg = stg_pool.tile([128, CH], fp32, name="ext_stg", tag="ex")
                    nc.scalar.activation(ext_stg[:], scat[:], AF.Ln, bias=epsb[:])
                    nc.sync.dma_start(ll_d[sl, b, V:TOT], ext_stg[:])

            if not kn["arc_early"]:
                emit_arc()

    _split_multi_waits(nc)
    return nc


_CACHE = {}


def _get_bass(cap):
    if cap not in _CACHE:
        _CACHE[cap] = _build_bass(cap)
    return _CACHE[cap]


def kernel(**inputs):
    in_maps, cap = _host_prep(inputs)
    nc = _get_bass(cap)
    res = run_bass_kernel_spmd(nc, in_maps, core_ids=list(range(NCORES)))
    ll = np.concatenate([res.results[i]["ll"] for i in range(NCORES)], axis=1)
    arcll = np.concatenate([res.results[i]["arcll"] for i in range(NCORES)], axis=1)
    return ll.astype(np.float32), arcll.astype(np.float32)
